# revision 1
# baseline (speedup 1.0000x reference)
"""CaiT (nn_Cait_78984448573778) forward on 8 trn2 NeuronCores.

Data-parallel over batch: each core runs the full model on 2 images.
Activations live transposed in SBUF ([C on partitions, tokens in free dim]).
All big matmuls in bf16 with fp32 PSUM accumulation.
Talking-heads pre-softmax mix is folded into the score matmul (per-output-head
scaled q, full-C contraction); the post-softmax mix runs as scaled-identity
PE matmuls that also transpose the attention maps for the AV matmul.
"""

from contextlib import ExitStack

import numpy as np
import ml_dtypes

import concourse.bass as bass
import concourse.mybir as mybir
import concourse.tile as tile
from concourse import bacc
from concourse.bass_utils import run_bass_kernel_spmd
from concourse.masks import make_identity

F32 = mybir.dt.float32
BF16 = mybir.dt.bfloat16
AF = mybir.ActivationFunctionType
MULT = mybir.AluOpType.mult

# model dims
B, C, DEPTH, HEADS, D2, NCLS = 16, 768, 12, 12, 2, 1000
P_, IMG = 16, 224
GH = IMG // P_          # 14
NP = GH * GH            # 196
HD = C // HEADS         # 64
SCALE = HD ** -0.5
EPS = 1e-6

NCORES = 8
BL = B // NCORES        # 2 images per core
TC = BL * NP            # 392 token-columns, col = b*196 + n
CH = C // 128           # 6 c-chunks
FF = 4 * C              # 3072
FCH = FF // 128         # 24
TN = NP + 1             # 197 tokens with cls
TCA = BL * TN           # 394 cols, col = b*197 + t (t=0 is cls)
NCH = [(0, 128), (128, 68)]    # n-chunks of 196
TCH = [(0, 128), (128, 69)]    # tok-chunks of 197

# th_pack column offsets
OFF_QKB = 0       # 12 cols  (q part pre-scaled)
OFF_F1B = 12      # 24
OFF_N1W = 36      # 6 each from here
OFF_N1B = 42
OFF_N2W = 48
OFF_N2B = 54
OFF_G1 = 60
OFF_G1PB = 66
OFF_G2 = 72
OFF_G2F2B = 78
OFF_PLW = 84      # 72 cols: (g,kchunk) -> plw[g, 2k + p//64]
OFF_PLB = 156     # 12
OFF_PWW = 168     # 144: g*12+j
OFF_PWB = 312     # 12
PCOLS = 324

# ca_pack offsets
CA_QB = 0     # 6 (scaled)
CA_KB = 6
CA_N1W = 12
CA_N1B = 18
CA_N2W = 24
CA_N2B = 30
CA_G1 = 36
CA_G1PB = 42
CA_G2 = 48
CA_G2F2B = 54
CA_F1B = 60   # 24
CACOLS = 84


def _bf(x):
    return np.ascontiguousarray(np.asarray(x, np.float32)).astype(ml_dtypes.bfloat16)


def _f32(x):
    return np.ascontiguousarray(np.asarray(x, np.float32))


def _pack_cols(vecs_768):
    """list of [768] vectors -> [128, 6*len] (chunk-major per vector)."""
    cols = []
    for v in vecs_768:
        cols.append(np.asarray(v, np.float32).reshape(CH, 128).T)  # [128, 6]
    return np.concatenate(cols, axis=1)


def host_prep(inp):
    """Build all DRAM input arrays (shared across cores except pT)."""
    d = {}
    d['patch_wT'] = _bf(np.asarray(inp['patch_w'], np.float32).T)
    posT = np.asarray(inp['pos_embed'], np.float32)[0].T  # [768, 196]
    posb = np.concatenate([posT, posT], axis=1) + np.asarray(
        inp['patch_b'], np.float32)[:, None]
    d['posb'] = _f32(posb)  # [768, 392]
    d['clsT'] = _f32(np.tile(np.asarray(inp['cls_token'],
                                        np.float32)[0, 0][:, None], (1, BL)))

    qkvw = np.asarray(inp['qkvw'], np.float32)  # [12, 2304, 768]
    wqk = qkvw[:, :2 * C, :].copy()
    wqk[:, :C, :] *= SCALE
    d['wqkT'] = _bf(wqk.transpose(0, 2, 1))                 # [12, 768, 1536]
    d['wvT'] = _bf(qkvw[:, 2 * C:, :].transpose(0, 2, 1))   # [12, 768, 768]
    d['projwT'] = _bf(np.asarray(inp['projw'], np.float32).transpose(0, 2, 1))
    d['f1T'] = _bf(np.asarray(inp['f1w'], np.float32).transpose(0, 2, 1))
    d['f2T'] = _bf(np.asarray(inp['f2w'], np.float32).transpose(0, 2, 1))

    qkvb = np.asarray(inp['qkvb'], np.float32)
    g1 = np.asarray(inp['g1'], np.float32)
    g2 = np.asarray(inp['g2'], np.float32)
    projb = np.asarray(inp['projb'], np.float32)
    f2b = np.asarray(inp['f2b'], np.float32)
    plw = np.asarray(inp['plw'], np.float32)
    plb = np.asarray(inp['plb'], np.float32)
    pww = np.asarray(inp['pww'], np.float32)
    pwb = np.asarray(inp['pwb'], np.float32)

    packs = []
    for L in range(DEPTH):
        qb = qkvb[L, :C] * SCALE
        kb = qkvb[L, C:2 * C]
        qkb = np.concatenate([qb, kb]).reshape(12, 128).T  # [128, 12]
        f1b = np.asarray(inp['f1b'], np.float32)[L].reshape(FCH, 128).T
        lnp = _pack_cols([inp['n1w'][L], inp['n1b'][L], inp['n2w'][L],
                          inp['n2b'][L], g1[L], g1[L] * projb[L],
                          g2[L], g2[L] * f2b[L]])
        pl = np.zeros((128, 72), np.float32)
        for g in range(HEADS):
            for k in range(CH):
                pl[:64, g * 6 + k] = plw[L, g, 2 * k]
                pl[64:, g * 6 + k] = plw[L, g, 2 * k + 1]
        plbb = np.tile(plb[L][None, :], (128, 1))
        pwwb = np.tile(pww[L].reshape(1, 144), (128, 1))
        pwbb = np.tile(pwb[L][None, :], (128, 1))
        packs.append(np.concatenate([qkb, f1b, lnp, pl, plbb, pwwb, pwbb],
                                    axis=1))
    d['th_pack'] = _f32(np.stack(packs))  # [12, 128, 324]
    d['vb_bc'] = _f32(np.tile(qkvb[:, 2 * C:].reshape(DEPTH, 1, C),
                              (1, 128, 1)))  # [12, 128, 768]

    tq = np.asarray(inp['tqw'], np.float32) * SCALE
    tw = np.concatenate(
        [tq.transpose(0, 2, 1),
         np.asarray(inp['tkw'], np.float32).transpose(0, 2, 1),
         np.asarray(inp['tvw'], np.float32).transpose(0, 2, 1),
         np.asarray(inp['tprojw'], np.float32).transpose(0, 2, 1)],
        axis=2)  # [2, 768, 3072]: q | k | v | proj
    d['twT'] = _bf(tw)
    d['tf1T'] = _bf(np.asarray(inp['tf1w'], np.float32).transpose(0, 2, 1))
    d['tf2T'] = _bf(np.asarray(inp['tf2w'], np.float32).transpose(0, 2, 1))
    tg1 = np.asarray(inp['tg1'], np.float32)
    tg2 = np.asarray(inp['tg2'], np.float32)
    tprojb = np.asarray(inp['tprojb'], np.float32)
    tf2b = np.asarray(inp['tf2b'], np.float32)
    cps = []
    for L in range(D2):
        lnp = _pack_cols([np.asarray(inp['tqb'], np.float32)[L] * SCALE,
                          inp['tkb'][L], inp['tn1w'][L], inp['tn1b'][L],
                          inp['tn2w'][L], inp['tn2b'][L],
                          tg1[L], tg1[L] * tprojb[L],
                          tg2[L], tg2[L] * tf2b[L]])
        f1b = np.asarray(inp['tf1b'], np.float32)[L].reshape(FCH, 128).T
        cps.append(np.concatenate([lnp, f1b], axis=1))
    d['ca_pack'] = _f32(np.stack(cps))
    d['ca_vb_bc'] = _f32(np.tile(np.asarray(inp['tvb'], np.float32)
                                 .reshape(D2, 1, C), (1, 128, 1)))

    d['fin_pack'] = _f32(_pack_cols([inp['normw'], inp['normb']]))
    d['headwT'] = _bf(np.asarray(inp['headw'], np.float32).T)
    d['headb2'] = _f32(np.tile(np.asarray(inp['headb'], np.float32)[None, :],
                               (BL, 1)))
    return d


def host_pT(x_slice):
    """[2,3,224,224] -> bf16 [768, 392] patch matrix, col = b*196 + n."""
    p = np.asarray(x_slice, np.float32).reshape(BL, 3, GH, P_, GH, P_)
    p = p.transpose(0, 2, 4, 1, 3, 5).reshape(BL, NP, 3 * P_ * P_)
    return _bf(p.transpose(2, 0, 1).reshape(3 * P_ * P_, BL * NP))


INPUT_SPECS = [
    ('pT', [C, TC], BF16), ('posb', [C, TC], F32), ('clsT', [C, BL], F32),
    ('patch_wT', [C, C], BF16),
    ('wqkT', [DEPTH, C, 2 * C], BF16), ('wvT', [DEPTH, C, C], BF16),
    ('projwT', [DEPTH, C, C], BF16), ('f1T', [DEPTH, C, FF], BF16),
    ('f2T', [DEPTH, FF, C], BF16), ('th_pack', [DEPTH, 128, PCOLS], F32),
    ('vb_bc', [DEPTH, 128, C], F32),
    ('twT', [D2, C, 4 * C], BF16), ('tf1T', [D2, C, FF], BF16),
    ('tf2T', [D2, FF, C], BF16), ('ca_pack', [D2, 128, CACOLS], F32),
    ('ca_vb_bc', [D2, 128, C], F32),
    ('fin_pack', [128, 12], F32), ('headwT', [C, NCLS], BF16),
    ('headb2', [BL, NCLS], F32),
]


def build_program(depth=DEPTH, d2=D2, repeat=1):
    nc = bacc.Bacc("TRN2", target_bir_lowering=False, debug=False,
                   num_devices=NCORES)
    aps = {}
    for name, shape, dt in INPUT_SPECS:
        aps[name] = nc.dram_tensor(name, shape, dt, kind="ExternalInput").ap()
    out_ap = nc.dram_tensor("out", [BL, NCLS], F32, kind="ExternalOutput").ap()
    with tile.TileContext(nc) as tc:
        with ExitStack() as es:
            emit_kernel(es, tc, out_ap, aps, depth, d2, repeat)
    nc.compile()
    return nc


def emit_kernel(es, tc, out_ap, aps, depth, d2, repeat=1):
    nc = tc.nc
    pers = es.enter_context(tc.tile_pool(name='pers', bufs=1))
    wp = es.enter_context(tc.tile_pool(name='wp', bufs=1))
    wbig = es.enter_context(tc.tile_pool(name='wbig', bufs=1))
    ap_ = es.enter_context(tc.tile_pool(name='act', bufs=1))
    sq_p = es.enter_context(tc.tile_pool(name='sq', bufs=3))
    tmp = es.enter_context(tc.tile_pool(name='tmp', bufs=2))
    idp = es.enter_context(tc.tile_pool(name='idp', bufs=14))
    atp = es.enter_context(tc.tile_pool(name='atp', bufs=4))
    glp = es.enter_context(tc.tile_pool(name='glp', bufs=3))
    psA = es.enter_context(tc.tile_pool(name='psA', bufs=6, space="PSUM"))
    psB = es.enter_context(tc.tile_pool(name='psB', bufs=2, space="PSUM"))

    # ---- persistent tiles ----
    hT = pers.tile([128, CH, TC], F32, tag='hT')
    xext = pers.tile([128, CH, TCA], F32, tag='xext')
    clsT = pers.tile([128, CH, BL], F32, tag='clsT')
    ident_b = pers.tile([128, 128], BF16, tag='idb')
    ident_f = pers.tile([128, 128], F32, tag='idf')
    ones_col = pers.tile([128, 1], F32, tag='onec')
    ones_row = pers.tile([1, 128], F32, tag='oner')
    ones_bf = pers.tile([128, 1], BF16, tag='onebf')
    nc.vector.memset(ones_bf[:], 1.0)
    make_identity(nc, ident_b[:])
    make_identity(nc, ident_f[:])
    nc.vector.memset(ones_col[:], 1.0 / C)
    nc.vector.memset(ones_row[:], 1.0)

    def dma(dst, src):
        nc.sync.dma_start(out=dst, in_=src)

    # ---------- layernorm helpers (transposed layout) ----------
    def ln_stats(src_slices, cols):
        """Cross-partition LN stats. Returns psum tiles (Rb, Mb) [128, cols]:
        Rb = rstd broadcast, Mb = -mean*rstd broadcast."""
        s1 = psA.tile([1, cols], F32, tag='pa')
        s2 = psA.tile([1, cols], F32, tag='pa')
        for c in range(CH):
            nc.tensor.matmul(s1[:], ones_col[:], src_slices[c],
                             start=(c == 0), stop=(c == CH - 1))
        for c in range(CH):
            sq = sq_p.tile([128, cols], F32, tag='lnsq')
            nc.scalar.square(sq[:], src_slices[c])
            nc.tensor.matmul(s2[:], ones_col[:], sq[:],
                             start=(c == 0), stop=(c == CH - 1))
        rows = tmp.tile([1, 2, cols], F32, tag='lnrows')
        v_, r = rows[:, 0, :], rows[:, 1, :]
        nc.scalar.square(v_, s1[:])                        # mu^2
        nc.vector.tensor_sub(v_, s2[:], v_)                # var
        nc.vector.tensor_scalar_add(v_, v_, EPS)
        nc.scalar.activation(v_, v_, AF.Sqrt)              # sd
        nc.vector.reciprocal(r, v_)                        # rstd
        nc.vector.tensor_mul(v_, s1[:], r)
        nc.vector.tensor_scalar_mul(v_, v_, -1.0)          # -mu*rstd
        Rb = psB.tile([128, cols], F32, tag='pb')
        Mb = psB.tile([128, cols], F32, tag='pb')
        nc.tensor.matmul(Rb[:], ones_row[:], r, start=True, stop=True)
        nc.tensor.matmul(Mb[:], ones_row[:], v_, start=True, stop=True)
        return Rb, Mb

    def ln_to_bf16(src, cols, pk, w_off, b_off, out_tag):
        Rb, Mb = ln_stats([src[:, c, :] for c in range(CH)], cols)
        y = ap_.tile([128, CH, cols], BF16, tag=out_tag)
        for c in range(CH):
            t1 = tmp.tile([128, cols], F32, tag='lnt1')
            nc.vector.tensor_mul(t1[:], src[:, c, :], Rb[:])
            nc.vector.tensor_add(t1[:], t1[:], Mb[:])
            nc.scalar.activation(y[:, c, :], t1[:], AF.Identity,
                                 bias=pk[:, b_off + c:b_off + c + 1],
                                 scale=pk[:, w_off + c:w_off + c + 1])
        return y

    for _rep in range(repeat):
        # ---------- patch embed ----------
        pt = ap_.tile([128, CH, TC], BF16, tag='big_bf')
        pw = wp.tile([128, CH, C], BF16, tag='wv')
        for c in range(CH):
            dma(pt[:, c, :], aps['pT'][c * 128:(c + 1) * 128, :])
            dma(pw[:, c, :], aps['patch_wT'][c * 128:(c + 1) * 128, :])
        for m in range(CH):
            pm = psA.tile([128, TC], F32, tag='pa')
            for k in range(CH):
                nc.tensor.matmul(pm[:], pw[:, k, m * 128:(m + 1) * 128],
                                 pt[:, k, :], start=(k == 0), stop=(k == CH - 1))
            pb = tmp.tile([128, TC], F32, tag='lnt1')
            dma(pb[:], aps['posb'][m * 128:(m + 1) * 128, :])
            nc.vector.tensor_add(hT[:, m, :], pm[:], pb[:])

        # ---------- talking-heads blocks ----------
        for L in range(depth):
            pk = wp.tile([128, PCOLS], F32, tag='pk')
            dma(pk[:], aps['th_pack'][L])
            vbb = wp.tile([128, C], F32, tag='vbb')
            dma(vbb[:], aps['vb_bc'][L])

            y = ln_to_bf16(hT, TC, pk, OFF_N1W, OFF_N1B, 'y')

            # qk projection -> qk [128, 12, 392] bf16 (q chunks 0-5 prescaled)
            wqk = wp.tile([128, CH, 2 * C], BF16, tag='wqk')
            for k in range(CH):
                dma(wqk[:, k, :], aps['wqkT'][L, k * 128:(k + 1) * 128, :])
            qk = ap_.tile([128, 2 * CH, TC], BF16, tag='big_bf')
            for m in range(2 * CH):
                pm = psA.tile([128, TC], F32, tag='pa')
                for k in range(CH):
                    nc.tensor.matmul(pm[:], wqk[:, k, m * 128:(m + 1) * 128],
                                     y[:, k, :], start=(k == 0), stop=(k == CH - 1))
                nc.scalar.activation(qk[:, m, :], pm[:], AF.Identity,
                                     bias=pk[:, OFF_QKB + m:OFF_QKB + m + 1])

            # v in natural layout: slot (b*2+chunk): [tok<=128, 768]
            wv = wp.tile([128, CH, C], BF16, tag='wv')
            for k in range(CH):
                dma(wv[:, k, :], aps['wvT'][L, k * 128:(k + 1) * 128, :])
            vnat = ap_.tile([128, 2 * BL, C], BF16, tag='vnat')
            for b in range(BL):
                for ci, (noff, nsz) in enumerate(NCH):
                    for half in range(2):
                        pv = psA.tile([128, 384], F32, tag='pa')
                        for k in range(CH):
                            nc.tensor.matmul(
                                pv[:nsz, :],
                                y[:, k, b * NP + noff:b * NP + noff + nsz],
                                wv[:, k, half * 384:(half + 1) * 384],
                                start=(k == 0), stop=(k == CH - 1))
                        nc.vector.tensor_add(
                            vnat[:nsz, b * 2 + ci, half * 384:(half + 1) * 384],
                            pv[:nsz, :], vbb[:nsz, half * 384:(half + 1) * 384])

            # premixed scores + exp; E[chunk] [128, 12, 2, 196] bf16
            E = [ap_.tile([128, HEADS, BL, NP], BF16, tag=f'E{ci}',
                          name=f'E{ci}_{L}') for ci in range(2)]
            Z = [tmp.tile([128, HEADS * BL], F32, tag=f'Z{ci}',
                          name=f'Z{ci}_{L}') for ci in range(2)]
            rZ = [tmp.tile([128, HEADS * BL], F32, tag=f'rZ{ci}',
                           name=f'rZ{ci}_{L}') for ci in range(2)]
            for g in range(HEADS):
                sq = sq_p.tile([128, CH, TC], BF16, tag='sq')
                for k in range(CH):
                    nc.vector.tensor_scalar_mul(
                        sq[:, k, :], qk[:, k, :],
                        pk[:, OFF_PLW + g * 6 + k:OFF_PLW + g * 6 + k + 1])
                for b in range(BL):
                    for ci, (noff, nsz) in enumerate(NCH):
                        pm = psA.tile([128, NP], F32, tag='pa')
                        for k in range(CH):
                            nc.tensor.matmul(
                                pm[:nsz, :],
                                sq[:, k, b * NP + noff:b * NP + noff + nsz],
                                qk[:, CH + k, b * NP:(b + 1) * NP],
                                start=(k == 0), stop=(k == CH - 1))
                        nc.scalar.activation(
                            E[ci][:nsz, g, b, :], pm[:nsz, :], AF.Exp,
                            bias=pk[:nsz, OFF_PLB + g:OFF_PLB + g + 1],
                            accum_out=Z[ci][:nsz, g * BL + b:g * BL + b + 1])
            for ci, (noff, nsz) in enumerate(NCH):
                nc.vector.reciprocal(rZ[ci][:nsz, :], Z[ci][:nsz, :])
            for g in range(HEADS):
                for b in range(BL):
                    for ci, (noff, nsz) in enumerate(NCH):
                        nc.vector.tensor_scalar_mul(
                            E[ci][:nsz, g, b, :], E[ci][:nsz, g, b, :],
                            rZ[ci][:nsz, g * BL + b:g * BL + b + 1])

            # post-softmax mix (scaled-identity matmuls -> transposed A) + AV
            oT = ap_.tile([128, CH, TC], BF16, tag='oT')
            for g in range(HEADS):
                ids = []
                for j in range(HEADS):
                    idt = idp.tile([128, 128], BF16, tag='id')
                    col = OFF_PWW + g * HEADS + j
                    nc.vector.tensor_scalar_mul(idt[:], ident_b[:],
                                                pk[:, col:col + 1])
                    ids.append(idt)
                for b in range(BL):
                    at = atp.tile([128, 2, NP], BF16, tag='at')
                    for mi, (moff, msz) in enumerate(NCH):
                        pp = psA.tile([128, NP], F32, tag='pa')
                        for ci, (noff, nsz) in enumerate(NCH):
                            for j in range(HEADS):
                                nc.tensor.matmul(
                                    pp[:msz, noff:noff + nsz],
                                    E[ci][:nsz, j, b, moff:moff + msz],
                                    ids[j][:nsz, :nsz],
                                    start=(j == 0), stop=(j == HEADS - 1))
                        nc.scalar.activation(
                            at[:msz, mi, :], pp[:msz, :], AF.Identity,
                            bias=pk[:msz, OFF_PWB + g:OFF_PWB + g + 1])
                    po = psA.tile([64, NP], F32, tag='pa')
                    for mi, (moff, msz) in enumerate(NCH):
                        nc.tensor.matmul(
                            po[:], vnat[:msz, b * 2 + mi, g * 64:(g + 1) * 64],
                            at[:msz, mi, :], start=(mi == 0), stop=(mi == 1))
                    nc.scalar.copy(
                        oT[(g % 2) * 64:(g % 2) * 64 + 64, g // 2,
                           b * NP:(b + 1) * NP], po[:])

            # attn out projection + residual
            pj = wp.tile([128, CH, C], BF16, tag='proj')
            for k in range(CH):
                dma(pj[:, k, :], aps['projwT'][L, k * 128:(k + 1) * 128, :])
            for m in range(CH):
                pm = psA.tile([128, TC], F32, tag='pa')
                for k in range(CH):
                    nc.tensor.matmul(pm[:], pj[:, k, m * 128:(m + 1) * 128],
                                     oT[:, k, :], start=(k == 0), stop=(k == CH - 1))
                nc.vector.affine_then_add(
                    hT[:, m, :], pm[:], hT[:, m, :],
                    scale=pk[:, OFF_G1 + m:OFF_G1 + m + 1],
                    bias=pk[:, OFF_G1PB + m:OFF_G1PB + m + 1])

            # MLP
            y2 = ln_to_bf16(hT, TC, pk, OFF_N2W, OFF_N2B, 'y')
            f1 = wbig.tile([128, CH, FF], BF16, tag='wbig')
            for k in range(CH):
                dma(f1[:, k, :], aps['f1T'][L, k * 128:(k + 1) * 128, :])
            gl = ap_.tile([128, FCH, TC], BF16, tag='gl')
            for kf in range(FCH):
                pm = psA.tile([128, TC], F32, tag='pa')
                for k in range(CH):
                    nc.tensor.matmul(pm[:], f1[:, k, kf * 128:(kf + 1) * 128],
                                     y2[:, k, :], start=(k == 0), stop=(k == CH - 1))
                nc.scalar.activation(gl[:, kf, :], pm[:], AF.Gelu,
                                     bias=pk[:, OFF_F1B + kf:OFF_F1B + kf + 1])
            f2 = wbig.tile([128, FCH, C], BF16, tag='wbig')
            for k in range(FCH):
                dma(f2[:, k, :], aps['f2T'][L, k * 128:(k + 1) * 128, :])
            for m in range(CH):
                pm = psA.tile([128, TC], F32, tag='pa')
                for kf in range(FCH):
                    nc.tensor.matmul(pm[:], f2[:, kf, m * 128:(m + 1) * 128],
                                     gl[:, kf, :], start=(kf == 0),
                                     stop=(kf == FCH - 1))
                nc.vector.affine_then_add(
                    hT[:, m, :], pm[:], hT[:, m, :],
                    scale=pk[:, OFF_G2 + m:OFF_G2 + m + 1],
                    bias=pk[:, OFF_G2F2B + m:OFF_G2F2B + m + 1])

        # ---------- x-hat for frozen h (into xext cols per image) ----------
        Rb, Mb = ln_stats([hT[:, c, :] for c in range(CH)], TC)
        for c in range(CH):
            t1 = tmp.tile([128, TC], F32, tag='lnt1')
            nc.vector.tensor_mul(t1[:], hT[:, c, :], Rb[:])
            for b in range(BL):
                nc.vector.tensor_add(
                    xext[:, c, b * TN + 1:(b + 1) * TN],
                    t1[:, b * NP:(b + 1) * NP], Mb[:, b * NP:(b + 1) * NP])

        for c in range(CH):
            dma(clsT[:, c, :], aps['clsT'][c * 128:(c + 1) * 128, :])

        # ---------- class-attention blocks ----------
        def cls_xhat():
            """Returns list of CH xhat tiles slices writer: yields (c, t1 AP)."""
            Rb, Mb = ln_stats([clsT[:, c, :] for c in range(CH)], BL)
            outs = []
            for c in range(CH):
                t1 = tmp.tile([128, BL], F32, tag=f'ct1_{c}', name=f'ct1_{c}')
                nc.vector.tensor_mul(t1[:], clsT[:, c, :], Rb[:])
                nc.vector.tensor_add(t1[:], t1[:], Mb[:])
                outs.append(t1)
            return outs

        for l in range(d2):
            cap = wp.tile([128, CACOLS], F32, tag='pk')
            dma(cap[:], aps['ca_pack'][l])
            cvb = wp.tile([128, C], F32, tag='vbb')
            dma(cvb[:], aps['ca_vb_bc'][l])
            for c, t1 in enumerate(cls_xhat()):
                for b in range(BL):
                    nc.vector.tensor_copy(xext[:, c, b * TN:b * TN + 1],
                                          t1[:, b:b + 1])
            u = ap_.tile([128, CH, TCA], BF16, tag='big_bf')
            for c in range(CH):
                nc.scalar.activation(u[:, c, :], xext[:, c, :], AF.Identity,
                                     bias=cap[:, CA_N1B + c:CA_N1B + c + 1],
                                     scale=cap[:, CA_N1W + c:CA_N1W + c + 1])
            tw = wbig.tile([128, CH, 4 * C], BF16, tag='wbig')
            for k in range(CH):
                dma(tw[:, k, :], aps['twT'][l, k * 128:(k + 1) * 128, :])
            # k-projection (transposed layout) with bias
            kv = ap_.tile([128, CH, TCA], BF16, tag='kv')
            for m in range(CH):
                pm = psA.tile([128, TCA], F32, tag='pa')
                for k in range(CH):
                    nc.tensor.matmul(pm[:], tw[:, k, C + m * 128:C + (m + 1) * 128],
                                     u[:, k, :], start=(k == 0), stop=(k == CH - 1))
                nc.scalar.activation(kv[:, m, :], pm[:], AF.Identity,
                                     bias=cap[:, CA_KB + m:CA_KB + m + 1])
            # v natural
            vnat = ap_.tile([128, 2 * BL, C], BF16, tag='vnat')
            for b in range(BL):
                for ci, (toff, tsz) in enumerate(TCH):
                    for half in range(2):
                        pv = psA.tile([128, 384], F32, tag='pa')
                        for k in range(CH):
                            nc.tensor.matmul(
                                pv[:tsz, :],
                                u[:, k, b * TN + toff:b * TN + toff + tsz],
                                tw[:, k, 2 * C + half * 384:2 * C + (half + 1) * 384],
                                start=(k == 0), stop=(k == CH - 1))
                        nc.vector.tensor_add(
                            vnat[:tsz, b * 2 + ci, half * 384:(half + 1) * 384],
                            pv[:tsz, :], cvb[:tsz, half * 384:(half + 1) * 384])
            # q (cls cols only) -> qT [128, CH, BL]
            qT = ap_.tile([128, CH, BL], BF16, tag='qT')
            for m in range(CH):
                pm = psA.tile([128, BL], F32, tag='pa')
                for b in range(BL):
                    for k in range(CH):
                        nc.tensor.matmul(
                            pm[:, b:b + 1], tw[:, k, m * 128:(m + 1) * 128],
                            u[:, k, b * TN:b * TN + 1],
                            start=(k == 0), stop=(k == CH - 1))
                nc.scalar.activation(qT[:, m, :], pm[:], AF.Identity,
                                     bias=cap[:, CA_QB + m:CA_QB + m + 1])
            # scores rows [1,197] -> exp -> transpose into Ecol [tok, 24] bf16
            Ecol = [ap_.tile([128, HEADS * BL], BF16, tag=f'Ec{ci}',
                             name=f'Ecol{ci}_{l}') for ci in range(2)]
            Zrow = tmp.tile([1, HEADS * BL], F32, tag='Zrow')
            for b in range(BL):
                for h in range(HEADS):
                    r = b * HEADS + h
                    pm = psA.tile([1, TN], F32, tag='pa')
                    rows = slice((h % 2) * 64, (h % 2) * 64 + 64)
                    nc.tensor.matmul(pm[:, 0:1], qT[rows, h // 2, b:b + 1],
                                     kv[rows, h // 2, b * TN:b * TN + 1],
                                     start=True, stop=True)
                    nc.tensor.matmul(pm[:, 1:TN], qT[rows, h // 2, b:b + 1],
                                     kv[rows, h // 2, b * TN + 1:(b + 1) * TN],
                                     start=True, stop=True)
                    erow = tmp.tile([1, TN], F32, tag='erow')
                    nc.scalar.activation(erow[:], pm[:], AF.Exp,
                                         accum_out=Zrow[:, r:r + 1])
                    for ci, (toff, tsz) in enumerate(TCH):
                        pt_ = psA.tile([128, 1], F32, tag='pa')
                        nc.tensor.transpose(pt_[:tsz, :],
                                            erow[:, toff:toff + tsz],
                                            ident_f[:1, :1])
                        nc.vector.tensor_copy(Ecol[ci][:tsz, r:r + 1],
                                              pt_[:tsz, :])
            zrow = tmp.tile([1, HEADS * BL], F32, tag='zrow_s')
            nc.vector.reciprocal(zrow[:], Zrow[:])
            pzb = psA.tile([64, HEADS * BL], F32, tag='pa')
            nc.tensor.matmul(pzb[:], ones_row[:1, :64], zrow[:],
                             start=True, stop=True)
            rzb = tmp.tile([64, HEADS * BL], F32, tag='rzb')
            nc.vector.tensor_copy(rzb[:], pzb[:])
            # AV + normalize -> oTc [128, CH, BL]
            oTc = ap_.tile([128, CH, BL], BF16, tag='qT')
            for b in range(BL):
                for h in range(HEADS):
                    r = b * HEADS + h
                    po = psA.tile([64, 1], F32, tag='pa')
                    for ci, (toff, tsz) in enumerate(TCH):
                        nc.tensor.matmul(po[:], vnat[:tsz, b * 2 + ci,
                                                     h * 64:(h + 1) * 64],
                                         Ecol[ci][:tsz, r:r + 1],
                                         start=(ci == 0), stop=(ci == 1))
                    nc.scalar.activation(
                        oTc[(h % 2) * 64:(h % 2) * 64 + 64, h // 2, b:b + 1],
                        po[:], AF.Identity, scale=rzb[:, r:r + 1])
            # proj + residual into clsT
            for m in range(CH):
                pm = psA.tile([128, BL], F32, tag='pa')
                for k in range(CH):
                    nc.tensor.matmul(
                        pm[:], tw[:, k, 3 * C + m * 128:3 * C + (m + 1) * 128],
                        oTc[:, k, :], start=(k == 0), stop=(k == CH - 1))
                nc.vector.affine_then_add(
                    clsT[:, m, :], pm[:], clsT[:, m, :],
                    scale=cap[:, CA_G1 + m:CA_G1 + m + 1],
                    bias=cap[:, CA_G1PB + m:CA_G1PB + m + 1])
            # cls MLP
            u2 = ap_.tile([128, CH, BL], BF16, tag='u2')
            for c, t1 in enumerate(cls_xhat()):
                nc.scalar.activation(u2[:, c, :], t1[:], AF.Identity,
                                     bias=cap[:, CA_N2B + c:CA_N2B + c + 1],
                                     scale=cap[:, CA_N2W + c:CA_N2W + c + 1])
            tf1 = wbig.tile([128, CH, FF], BF16, tag='wbig')
            for k in range(CH):
                dma(tf1[:, k, :], aps['tf1T'][l, k * 128:(k + 1) * 128, :])
            gl2 = ap_.tile([128, FCH, BL], BF16, tag='gl2')
            for kf in range(FCH):
                pm = psA.tile([128, BL], F32, tag='pa')
                for k in range(CH):
                    nc.tensor.matmul(pm[:], tf1[:, k, kf * 128:(kf + 1) * 128],
                                     u2[:, k, :], start=(k == 0), stop=(k == CH - 1))
                nc.scalar.activation(gl2[:, kf, :], pm[:], AF.Gelu,
                                     bias=cap[:, CA_F1B + kf:CA_F1B + kf + 1])
            tf2 = wbig.tile([128, FCH, C], BF16, tag='wbig')
            for k in range(FCH):
                dma(tf2[:, k, :], aps['tf2T'][l, k * 128:(k + 1) * 128, :])
            for m in range(CH):
                pm = psA.tile([128, BL], F32, tag='pa')
                for kf in range(FCH):
                    nc.tensor.matmul(pm[:], tf2[:, kf, m * 128:(m + 1) * 128],
                                     gl2[:, kf, :], start=(kf == 0),
                                     stop=(kf == FCH - 1))
                nc.vector.affine_then_add(
                    clsT[:, m, :], pm[:], clsT[:, m, :],
                    scale=cap[:, CA_G2 + m:CA_G2 + m + 1],
                    bias=cap[:, CA_G2F2B + m:CA_G2F2B + m + 1])

        # ---------- final LN (cls only) + head ----------
        finp = pers.tile([128, 12], F32, tag='finp')
        dma(finp[:], aps['fin_pack'])
        zf = ap_.tile([128, CH, BL], BF16, tag='u2')
        for c, t1 in enumerate(cls_xhat()):
            nc.scalar.activation(zf[:, c, :], t1[:], AF.Identity,
                                 bias=finp[:, 6 + c:6 + c + 1],
                                 scale=finp[:, c:c + 1])
        hw = wp.tile([128, CH, NCLS], BF16, tag='wqk')
        for k in range(CH):
            dma(hw[:, k, :], aps['headwT'][k * 128:(k + 1) * 128, :])
        hb = pers.tile([BL, NCLS], F32, tag='hb')
        dma(hb[:], aps['headb2'])
        out_t = pers.tile([BL, NCLS], F32, tag='outt')
        for nh in range(2):
            phd = psA.tile([BL, 500], F32, tag='pa')
            for k in range(CH):
                nc.tensor.matmul(phd[:], zf[:, k, :],
                                 hw[:, k, nh * 500:(nh + 1) * 500],
                                 start=(k == 0), stop=(k == CH - 1))
            nc.vector.tensor_add(out_t[:, nh * 500:(nh + 1) * 500], phd[:],
                                 hb[:, nh * 500:(nh + 1) * 500])
        dma(out_ap[:, :], out_t[:])


_NC_CACHE = {}


def kernel(**inputs):
    if 'full' not in _NC_CACHE:
        _NC_CACHE['full'] = build_program()
    nc = _NC_CACHE['full']
    shared = host_prep(inputs)
    x = np.asarray(inputs['x'], np.float32)
    in_maps = []
    for c in range(NCORES):
        m = dict(shared)
        m['pT'] = host_pT(x[c * BL:(c + 1) * BL])
        in_maps.append(m)
    res = run_bass_kernel_spmd(nc, in_maps, list(range(NCORES)))
    return np.concatenate([res.results[c]['out'] for c in range(NCORES)],
                          axis=0).astype(np.float32)


if __name__ == '__main__':
    import time
    t0 = time.time()
    build_program()
    print("traced+compiled ok in", time.time() - t0, "s")

